# revision 29
# baseline (speedup 1.0000x reference)
"""Trainium2 Bass kernel for nn_Net_91268055040039 (dense_mlp).

Computes out[b] = sum_{t,p} x[b,t,p] * |W[t,p]| * fc1_w[0, t*P+p] + fc1_b
  x: [32, 400, 10000] f32, W: [400, 10000] f32, fc1_w: [1, 4000000] f32.

Strategy: shard the reduction dim T=400 into 8 slices of 50 rows. The kernel
is HBM-bandwidth bound (512MB of x), so x is streamed as FP16 (half the
bytes; rel err ~2e-3 vs the 2e-2 gate - inputs are N(0,1), errors average
out over the 4M-term sum). v = |W|*fc1 is precomputed on the host, fp16.

The multiply+reduce runs on the TENSOR engine (DVE scalar_tensor_tensor
has no 2x fp16 uop - measured 4.2us per 3908-elem op = 135us total, the
bottleneck of an earlier version). Layout is k-major: partition p holds
k = n*128 + p, so PE contracts 128 k-values per matmul. To beat the
60-cycle-per-matmul floor, G=16 k-groups share one matmul via the diagonal
trick: lhsT = v[:, n:n+16] (16 cols, LDWEIGHTS ~13ns), rhs = x[:, (n,b)
block] [128, 16*32], accumulating psum[16, 512] (one full PSUM bank) over
all 245 groups; only the g==g' diagonal [g, g*32:(g+1)*32] is wanted (the
off-diag products are discarded on the host). PE streams 1 rhs row/cycle:
NJG*B rows * 0.42ns ~= 53us busy < ~80us DMA stream.

DMA: all chunks on the single sync/HWDGE queue, which streams descriptors
back-to-back at engine line rate (27 GB/s x 16 engines) at any run length.
The chunk schedule tapers geometrically so PE's remaining work always fits
inside DMA's remaining stream time - the compute tail past the final DMA
is ~2us. bufs=6 gives ~5 chunks of rotation slack so transient PE lag
(HBM congestion from co-tenants) does not stall the stream.

End per core: one scalar copy psum -> sbuf, DMA out [16, 512]. The host
extracts diagonal blocks, sums the 8 per-core partials and adds fc1_b.
"""

import numpy as np

import concourse.bass as bass
import concourse.bacc as bacc
import concourse.mybir as mybir
from concourse.tile import TileContext
from concourse.bass_utils import run_bass_kernel_spmd

B, T, P = 32, 400, 10000
NCORES = 8
TS = T // NCORES          # 50 T-rows per core
K = TS * P                # 500000 reduction elements per core per batch
PART = 128
G = 16                    # k-groups (of 128) packed per matmul
NJG = 3920                # ceil(K/128)=3907 rounded up to a multiple of G
KPAD = NJG * PART         # 501760 (1760 zero pad)
# n-groups per DMA (sum = NJG). Geometric taper: chunk i small enough that
# PE's remaining work (~13.5ns/group) fits inside DMA's remaining stream
# time (~19ns/group) plus a ~1.5us tail - keeps the tensor engine off the
# critical path at the end. Short per-partition runs are NOT slow on this
# HWDGE path (measured 26-27 GB/s/engine down to 1KB runs).
# Front chunks come from the big pool (bufs=5), tail chunks (<=208 groups)
# from their own small pool (bufs=4): in slow-HBM phases PE falls ~15us
# behind and the tail DMAs otherwise stall on big-buffer rotation.
CHUNKS = (448, 448, 448, 448, 448, 448, 432, 208, 208, 176, 112, 64, 32)
TAIL_MAX = 208
CHUNK_MAX = max(CHUNKS)   # 448
F32 = mybir.dt.float32
F16 = mybir.dt.float16

# Set by the test harness to capture an NTFF profile; harmless when False.
TRACE = False
LAST_RESULT = None


def build_program() -> bass.Bass:
    # Bacc (not raw Bass): its compile() splits multi-sem waits into separate
    # instructions - this neuronxcc build allows only 1 sync-wait per inst.
    nc = bacc.Bacc()
    xs = nc.declare_dram_parameter("xs", [PART, NJG * B], F16, isOutput=False)
    vd = nc.declare_dram_parameter("vd", [PART, NJG], F16, isOutput=False)
    out = nc.declare_dram_parameter("out", [G, G * B], F32, isOutput=True)

    with TileContext(nc) as tc:
        with (
            tc.tile_pool(name="const", bufs=1) as cpool,
            tc.tile_pool(name="xp", bufs=5) as xpool,
            tc.tile_pool(name="xt2", bufs=4) as tpool,
            tc.tile_pool(name="psum", bufs=1, space="PSUM") as ppool,
        ):
            # v first on the same sync/HWDGE ring as x (2.3us for 1MB).
            # Any cross-ring placement (scalar HWDGE or gpsimd SWDGE) makes
            # the engines round-robin v's packets against the x stream, which
            # inflates per-engine busy time ~3us and delays x's first byte -
            # the 16 SDMA engines are the saturated resource, so moving bytes
            # between rings never helps, it only perturbs completion order.
            vt = cpool.tile([PART, NJG], F16)
            nc.sync.dma_start(out=vt, in_=vd[:, :])

            ps = ppool.tile([G, G * B], F32)
            nmm = NJG // G
            mm = 0
            n0 = 0
            for cn in CHUNKS:
                if cn > TAIL_MAX:
                    xt = xpool.tile([PART, CHUNK_MAX * B], F16, tag="xt")
                else:
                    xt = tpool.tile([PART, TAIL_MAX * B], F16, tag="xtt")
                # All x chunks on the single sync/HWDGE ring. Every cross-ring
                # variant measured worse: alternating HWDGE rings +22us,
                # chunk-0-on-gpsimd +3us, v-on-scalar +3us - engines
                # round-robin between queues at packet granularity, which
                # delays individual chunk completions and starves the PE.
                nc.sync.dma_start(
                    out=xt[:, : cn * B], in_=xs[:, n0 * B : (n0 + cn) * B]
                )
                for i in range(cn // G):
                    nc.tensor.matmul(
                        out=ps,
                        lhsT=vt[:, n0 + i * G : n0 + (i + 1) * G],
                        rhs=xt[:, i * G * B : (i + 1) * G * B],
                        start=(mm == 0),
                        stop=(mm == nmm - 1),
                    )
                    mm += 1
                n0 += cn

            # Ship the whole [G, G*B] accumulator; the host extracts the
            # diagonal blocks psum[g, g*B:(g+1)*B] and sums over g (8KB out).
            res = cpool.tile([G, G * B], F32)
            nc.scalar.copy(res, ps)
            nc.sync.dma_start(out=out[:, :], in_=res)
    nc.finalize()
    return nc


def make_in_maps(x: np.ndarray, W: np.ndarray, fc1_w: np.ndarray):
    x = np.asarray(x)
    v_full = np.abs(np.asarray(W, dtype=np.float32)) * np.asarray(
        fc1_w, dtype=np.float32
    ).reshape(T, P)
    in_maps = []
    for c in range(NCORES):
        t0 = c * TS
        # x k-major: xs[p, n*B + b] = x[b, k=n*128+p]
        xpad = np.zeros((B, KPAD), dtype=np.float16)
        xpad[:, :K] = x[:, t0 : t0 + TS, :].reshape(B, K)
        xs = np.ascontiguousarray(
            xpad.reshape(B, NJG, PART).transpose(2, 1, 0)
        ).reshape(PART, NJG * B)
        vpad = np.zeros(KPAD, dtype=np.float16)
        vpad[:K] = v_full[t0 : t0 + TS].reshape(-1)
        vs = np.ascontiguousarray(vpad.reshape(NJG, PART).T)
        in_maps.append({"xs": xs, "vd": vs})
    return in_maps


def kernel(x, W, fc1_w, fc1_b):
    global LAST_RESULT
    nc = build_program()
    in_maps = make_in_maps(x, W, fc1_w)
    res = run_bass_kernel_spmd(
        nc, in_maps, core_ids=list(range(NCORES)), trace=TRACE
    )
    LAST_RESULT = res
    partial = np.zeros(B, dtype=np.float64)
    for r in res.results:
        o = r["out"].astype(np.float64)          # [G, G*B]
        for g in range(G):
            partial += o[g, g * B : (g + 1) * B]
    out = partial.astype(np.float32) + np.float32(np.asarray(fc1_b).reshape(-1)[0])
    return out.reshape(B, 1).astype(np.float32)


# revision 30
# speedup vs baseline: 1.0429x; 1.0429x over previous
"""Trainium2 Bass kernel for nn_Net_91268055040039 (dense_mlp).

Computes out[b] = sum_{t,p} x[b,t,p] * |W[t,p]| * fc1_w[0, t*P+p] + fc1_b
  x: [32, 400, 10000] f32, W: [400, 10000] f32, fc1_w: [1, 4000000] f32.

Strategy: shard the reduction dim T=400 into 8 slices of 50 rows. The kernel
is HBM-bandwidth bound (512MB of x), so x is streamed as FP16 (half the
bytes; rel err ~2e-3 vs the 2e-2 gate - inputs are N(0,1), errors average
out over the 4M-term sum). v = |W|*fc1 is precomputed on the host, fp16.

The multiply+reduce runs on the TENSOR engine (DVE scalar_tensor_tensor
has no 2x fp16 uop - measured 4.2us per 3908-elem op = 135us total, the
bottleneck of an earlier version). Layout is k-major: partition p holds
k = n*128 + p, so PE contracts 128 k-values per matmul. To beat the
60-cycle-per-matmul floor, G=16 k-groups share one matmul via the diagonal
trick: lhsT = v[:, n:n+16] (16 cols, LDWEIGHTS ~13ns), rhs = x[:, (n,b)
block] [128, 16*32], accumulating psum[16, 512] (one full PSUM bank) over
all 245 groups; only the g==g' diagonal [g, g*32:(g+1)*32] is wanted (the
off-diag products are discarded on the host). PE streams 1 rhs row/cycle:
NJG*B rows * 0.42ns ~= 53us busy < ~80us DMA stream.

DMA: all chunks on the single sync/HWDGE queue, which streams descriptors
back-to-back at engine line rate (27 GB/s x 16 engines) at any run length.
The chunk schedule tapers geometrically so PE's remaining work always fits
inside DMA's remaining stream time - the compute tail past the final DMA
is ~2us. bufs=6 gives ~5 chunks of rotation slack so transient PE lag
(HBM congestion from co-tenants) does not stall the stream.

End per core: one scalar copy psum -> sbuf, DMA out [16, 512]. The host
extracts diagonal blocks, sums the 8 per-core partials and adds fc1_b.
"""

import numpy as np

import concourse.bass as bass
import concourse.bacc as bacc
import concourse.mybir as mybir
from concourse.tile import TileContext
from concourse.bass_utils import run_bass_kernel_spmd

B, T, P = 32, 400, 10000
NCORES = 8
TS = T // NCORES          # 50 T-rows per core
K = TS * P                # 500000 reduction elements per core per batch
PART = 128
G = 16                    # k-groups (of 128) packed per matmul
NJG = 3920                # ceil(K/128)=3907 rounded up to a multiple of G
KPAD = NJG * PART         # 501760 (1760 zero pad)
# n-groups per DMA (sum = NJG). Geometric taper: chunk i small enough that
# PE's remaining work (measured 15.3ns/group fast, 17.1 congested) fits
# inside DMA's remaining stream time (~19ns/group) plus a ~1.2us tail -
# keeps the tensor engine off the critical path at the end. Short
# per-partition runs are NOT slow on this HWDGE path (measured 26-27
# GB/s/engine down to 1KB runs).
# Front chunks come from the big pool (bufs=5), tail chunks (<=208 groups)
# from their own small pool (bufs=4): in slow-HBM phases PE falls ~15us
# behind and the tail DMAs otherwise stall on big-buffer rotation.
CHUNKS = (448, 448, 448, 448, 448, 448, 432, 192, 160, 128, 96, 80, 64, 48, 32)
TAIL_MAX = 192
CHUNK_MAX = max(CHUNKS)   # 448
F32 = mybir.dt.float32
F16 = mybir.dt.float16

# Set by the test harness to capture an NTFF profile; harmless when False.
TRACE = False
LAST_RESULT = None


def build_program() -> bass.Bass:
    # Bacc (not raw Bass): its compile() splits multi-sem waits into separate
    # instructions - this neuronxcc build allows only 1 sync-wait per inst.
    nc = bacc.Bacc()
    xs = nc.declare_dram_parameter("xs", [PART, NJG * B], F16, isOutput=False)
    vd = nc.declare_dram_parameter("vd", [PART, NJG], F16, isOutput=False)
    out = nc.declare_dram_parameter("out", [G, G * B], F32, isOutput=True)

    with TileContext(nc) as tc:
        with (
            tc.tile_pool(name="const", bufs=1) as cpool,
            tc.tile_pool(name="xp", bufs=5) as xpool,
            tc.tile_pool(name="xt2", bufs=4) as tpool,
            tc.tile_pool(name="psum", bufs=1, space="PSUM") as ppool,
        ):
            # v first on the same sync/HWDGE ring as x (2.3us for 1MB).
            # Any cross-ring placement (scalar HWDGE or gpsimd SWDGE) makes
            # the engines round-robin v's packets against the x stream, which
            # inflates per-engine busy time ~3us and delays x's first byte -
            # the 16 SDMA engines are the saturated resource, so moving bytes
            # between rings never helps, it only perturbs completion order.
            vt = cpool.tile([PART, NJG], F16)
            nc.sync.dma_start(out=vt, in_=vd[:, :])

            ps = ppool.tile([G, G * B], F32)
            nmm = NJG // G
            mm = 0
            n0 = 0
            for cn in CHUNKS:
                if cn > TAIL_MAX:
                    xt = xpool.tile([PART, CHUNK_MAX * B], F16, tag="xt")
                else:
                    xt = tpool.tile([PART, TAIL_MAX * B], F16, tag="xtt")
                # All x chunks on the single sync/HWDGE ring. Every cross-ring
                # variant measured worse: alternating HWDGE rings +22us,
                # chunk-0-on-gpsimd +3us, v-on-scalar +3us - engines
                # round-robin between queues at packet granularity, which
                # delays individual chunk completions and starves the PE.
                nc.sync.dma_start(
                    out=xt[:, : cn * B], in_=xs[:, n0 * B : (n0 + cn) * B]
                )
                for i in range(cn // G):
                    nc.tensor.matmul(
                        out=ps,
                        lhsT=vt[:, n0 + i * G : n0 + (i + 1) * G],
                        rhs=xt[:, i * G * B : (i + 1) * G * B],
                        start=(mm == 0),
                        stop=(mm == nmm - 1),
                    )
                    mm += 1
                n0 += cn

            # Ship the whole [G, G*B] accumulator; the host extracts the
            # diagonal blocks psum[g, g*B:(g+1)*B] and sums over g (8KB out).
            res = cpool.tile([G, G * B], F32)
            nc.scalar.copy(res, ps)
            nc.sync.dma_start(out=out[:, :], in_=res)
    nc.finalize()
    return nc


def make_in_maps(x: np.ndarray, W: np.ndarray, fc1_w: np.ndarray):
    x = np.asarray(x)
    v_full = np.abs(np.asarray(W, dtype=np.float32)) * np.asarray(
        fc1_w, dtype=np.float32
    ).reshape(T, P)
    in_maps = []
    for c in range(NCORES):
        t0 = c * TS
        # x k-major: xs[p, n*B + b] = x[b, k=n*128+p]
        xpad = np.zeros((B, KPAD), dtype=np.float16)
        xpad[:, :K] = x[:, t0 : t0 + TS, :].reshape(B, K)
        xs = np.ascontiguousarray(
            xpad.reshape(B, NJG, PART).transpose(2, 1, 0)
        ).reshape(PART, NJG * B)
        vpad = np.zeros(KPAD, dtype=np.float16)
        vpad[:K] = v_full[t0 : t0 + TS].reshape(-1)
        vs = np.ascontiguousarray(vpad.reshape(NJG, PART).T)
        in_maps.append({"xs": xs, "vd": vs})
    return in_maps


def kernel(x, W, fc1_w, fc1_b):
    global LAST_RESULT
    nc = build_program()
    in_maps = make_in_maps(x, W, fc1_w)
    res = run_bass_kernel_spmd(
        nc, in_maps, core_ids=list(range(NCORES)), trace=TRACE
    )
    LAST_RESULT = res
    partial = np.zeros(B, dtype=np.float64)
    for r in res.results:
        o = r["out"].astype(np.float64)          # [G, G*B]
        for g in range(G):
            partial += o[g, g * B : (g + 1) * B]
    out = partial.astype(np.float32) + np.float32(np.asarray(fc1_b).reshape(-1)[0])
    return out.reshape(B, 1).astype(np.float32)
